# revision 1
# baseline (speedup 1.0000x reference)
"""Confidence-histogram (ECE bin stats) Trainium2 Bass kernel.

Full-input contract: kernel(logits[1M,128] f32, labels[1M] int) -> [15,2] f32.

Math: conf = max(softmax(x)) = exp(max(x)) / sum(exp(x)); prediction is
correct iff x[label] == max(x) (ties are measure-zero for randn inputs).
Binning is done with cumulative threshold counts in log space:
    t = max(x) - ln(sum(exp(x)));   conf >= b/15  <=>  t >= ln(b/15)
Each core computes, for b in 1..14, ct_b = #{t >= theta_b} and
cc_b = #{t >= theta_b and correct}, plus cc_0 = total correct, as
per-partition partial sums. Host diffs the cumulative counts into the
[15,2] (correct, incorrect) table.

Sharding: data-parallel over N across 8 cores; each 125k-sample shard is
padded to 128*992 rows laid out partition-major so every DMA descriptor
reads 8KB contiguous HBM. Pad rows are [1,0,...,0] with label-logit 0:
conf=e/(e+127)~0.021 < 1/15 so they never cross any threshold, and they
are never counted correct, making them invisible to the output.
"""

import numpy as np

import concourse.bass as bass
import concourse.bacc as bacc
import concourse.tile as tile
from concourse import mybir
from concourse.bass_utils import run_bass_kernel_spmd

N_BINS = 15
C = 128
N_CORES = 8
P = 128          # SBUF partitions
Q = 992          # samples per partition per core (padded)
N_PAD = P * Q    # 126976 padded samples per core
CH = 16          # 128-sample slices per chunk (1MB DMA)
N_CHUNKS = Q // CH  # 62

_F32 = mybir.dt.float32


def _build_bass(q: int = Q, ch: int = CH) -> bass.Bass:
    n_pad = P * q
    n_chunks = q // ch
    nc = bacc.Bacc(None, target_bir_lowering=False)
    lg = nc.dram_tensor("logits", [n_pad, C], _F32, kind="ExternalInput")
    xl = nc.dram_tensor("xl", [P, q], _F32, kind="ExternalInput")
    out = nc.dram_tensor("out", [P, 32], _F32, kind="ExternalOutput")

    # partition-major view: sample (p, q) lives at HBM row p*Q + q
    lgv = lg[:].rearrange("(p q) c -> p q c", p=P)

    # thresholds ln(b/15) computed from the same f32 linspace the reference uses
    lowers = np.linspace(0.0, 1.0, N_BINS + 1, dtype=np.float32)[:-1]
    thetas = [float(np.log(np.float64(lowers[b]))) for b in range(1, N_BINS)]

    with tile.TileContext(nc) as tc:
        with (
            tc.tile_pool(name="xin", bufs=3) as xpool,
            tc.tile_pool(name="eexp", bufs=3) as epool,
            tc.tile_pool(name="jnk", bufs=2) as jpool,
            tc.tile_pool(name="wide", bufs=1) as wide,
        ):
            me = wide.tile([P, q], _F32)     # per-sample max(exp(x)) = exp(max(x))
            sr = wide.tile([P, q], _F32)     # per-sample sum(exp(x))
            xlt = wide.tile([P, q], _F32)    # per-sample x[label]
            tt = wide.tile([P, q], _F32)     # t = ln(me) - ln(sr)
            accv = wide.tile([P, q], _F32)   # correctness 0/1
            mask = wide.tile([P, q], _F32)   # threshold mask scratch
            junk = wide.tile([P, q], _F32)   # scratch
            out_t = wide.tile([P, 32], _F32)

            nc.vector.memset(out_t[:], 0.0)
            nc.sync.dma_start(out=xlt[:], in_=xl[:])

            for j in range(n_chunks):
                xt = xpool.tile([P, ch, C], _F32)
                nc.sync.dma_start(out=xt[:], in_=lgv[:, j * ch : (j + 1) * ch, :])
                et = epool.tile([P, ch, C], _F32)
                # first m slices: ACT computes exp AND the per-sample sum via
                # its accumulator, offloading those sums from the DVE; the
                # rest are exp'd in one bulk op and summed on the DVE.
                m = min(4, ch)  # ACT-offloaded sums per chunk (cost-model optimum)
                for k in range(m):
                    col = j * ch + k
                    nc.scalar.activation(
                        out=et[:, k, :], in_=xt[:, k, :],
                        func=mybir.ActivationFunctionType.Exp,
                        accum_out=sr[:, col : col + 1],
                    )
                if m < ch:
                    nc.scalar.activation(
                        out=et[:, m:ch, :], in_=xt[:, m:ch, :],
                        func=mybir.ActivationFunctionType.Exp,
                    )
                jt = jpool.tile([P, ch, C], _F32)
                # per-slice tensor_scalar+accum runs in 2x DVE mode (vs 1x for
                # tensor_reduce): op1 is the accumulator's reduce op
                for k in range(ch):
                    col = j * ch + k
                    nc.vector.tensor_scalar(
                        jt[:, k, :], et[:, k, :], 0.0, None, mybir.AluOpType.add,
                        op1=mybir.AluOpType.max, accum_out=me[:, col : col + 1],
                    )
                    if k >= m:
                        nc.vector.tensor_scalar(
                            jt[:, k, :], et[:, k, :], 1.0, None, mybir.AluOpType.mult,
                            op1=mybir.AluOpType.add, accum_out=sr[:, col : col + 1],
                        )

            # ---- per-sample epilogue on [128, Q] wide tiles ----
            # t = ln(max e) - ln(sum e)  (log-confidence)
            nc.scalar.activation(
                out=tt[:], in_=me[:], func=mybir.ActivationFunctionType.Ln
            )
            nc.scalar.activation(
                out=junk[:], in_=sr[:], func=mybir.ActivationFunctionType.Ln
            )
            nc.vector.tensor_tensor(tt[:], tt[:], junk[:], mybir.AluOpType.subtract)
            # acc: exp(x[label]) == max(exp(x)), with exp computed on-device so
            # rounding matches the per-class exps exactly
            nc.scalar.activation(
                out=mask[:], in_=xlt[:], func=mybir.ActivationFunctionType.Exp
            )
            nc.vector.tensor_tensor(accv[:], mask[:], me[:], mybir.AluOpType.is_equal)
            # col 16: total correct
            nc.vector.tensor_scalar(
                junk[:], accv[:], 1.0, None, mybir.AluOpType.mult,
                op1=mybir.AluOpType.add, accum_out=out_t[:, 16:17],
            )
            for b in range(1, N_BINS):
                # col b: ct_b = sum(t >= theta_b); op1 is the accum reduce op
                nc.vector.tensor_scalar(
                    mask[:], tt[:], thetas[b - 1], None, mybir.AluOpType.is_ge,
                    op1=mybir.AluOpType.add,
                    accum_out=out_t[:, b : b + 1],
                )
                # col 16+b: cc_b = sum((t >= theta_b) * acc) in one fused op
                nc.vector.scalar_tensor_tensor(
                    out=junk[:], in0=tt[:], scalar=thetas[b - 1], in1=accv[:],
                    op0=mybir.AluOpType.is_ge, op1=mybir.AluOpType.mult,
                    accum_out=out_t[:, 16 + b : 17 + b],
                )
            nc.sync.dma_start(out=out[:], in_=out_t[:])
    nc.compile()
    return nc


_NC_CACHE = None


def _get_nc() -> bass.Bass:
    global _NC_CACHE
    if _NC_CACHE is None:
        _NC_CACHE = _build_bass()
    return _NC_CACHE


def make_in_maps(logits: np.ndarray, labels: np.ndarray):
    """Shard + pad full inputs into per-core input maps."""
    n = logits.shape[0]
    n_per = n // N_CORES
    assert n_per * N_CORES == n and n_per <= N_PAD
    idx = np.arange(n, dtype=np.int64)
    xl_full = logits[idx, labels.astype(np.int64)].astype(np.float32)

    pad_row = np.zeros(C, np.float32)
    pad_row[0] = 1.0
    in_maps = []
    for s in range(N_CORES):
        lo = s * n_per
        shard = np.empty((N_PAD, C), np.float32)
        shard[:n_per] = logits[lo : lo + n_per]
        shard[n_per:] = pad_row  # conf ~0.021 (bin 0), never correct
        xls = np.zeros(N_PAD, np.float32)
        xls[:n_per] = xl_full[lo : lo + n_per]
        in_maps.append({"logits": shard, "xl": xls.reshape(P, Q)})
    return in_maps


def combine_outputs(results, n: int) -> np.ndarray:
    """Fold per-core [128,32] partials into the [15,2] bin_stats table."""
    ct = np.zeros(N_BINS + 1, np.float64)  # cumulative totals, index b
    cc = np.zeros(N_BINS + 1, np.float64)  # cumulative corrects
    ct[0] = float(n)
    for r in results:
        o = np.asarray(r["out"], np.float64)
        colsum = o.sum(axis=0)
        ct[1:N_BINS] += colsum[1:N_BINS]
        cc[0] += colsum[16]
        cc[1:N_BINS] += colsum[17 : 16 + N_BINS]
    total = ct[:-1] - ct[1:]
    correct = cc[:-1] - cc[1:]
    return np.stack([correct, total - correct], axis=1).astype(np.float32)


def kernel(logits, labels) -> np.ndarray:
    logits = np.asarray(logits, dtype=np.float32)
    labels = np.asarray(labels)
    n = logits.shape[0]
    in_maps = make_in_maps(logits, labels)
    res = run_bass_kernel_spmd(_get_nc(), in_maps, core_ids=list(range(N_CORES)))
    return combine_outputs(res.results, n)



# revision 2
# speedup vs baseline: 1.9848x; 1.9848x over previous
"""Confidence-histogram (ECE bin stats) Trainium2 Bass kernel.

Full-input contract: kernel(logits[1M,128] f32, labels[1M] int) -> [15,2] f32.

Math: conf = max(softmax(x)) = exp(mx) / S with mx = max_c x_c, S = sum_c
exp(x_c); prediction is correct iff x[label] == mx (ties are measure-zero
for randn inputs).  Binning via cumulative threshold counts in log space:
    t = mx - ln(S);   conf >= b/15  <=>  t >= ln(b/15)
Each core computes, for b in 1..14, ct_b = #{t >= theta_b} and
cc_b = #{t >= theta_b and correct}, plus cc_0 = total correct, as
per-partition partial sums. Host diffs the cumulative counts into the
[15,2] (correct, incorrect) table.

Inputs are cast to fp16 on the host (rel-err impact ~7e-5, gate is 2e-2),
which halves HBM traffic and unlocks the DVE 2x_1p mode for the pairwise
fold tree that replaces 1x per-sample reductions:
    max: [P,ch,128] -fold-> 64 -> 32 -> 16 -> 8 -then-> tensor_reduce(X)
    sum: same tree over exp(x) (ACT bulk exp, fp16 out)
Sharding: data-parallel over N across 8 cores; each 125k-sample shard is
padded to 128*992 rows laid out partition-major so every DMA descriptor
reads ~16KB contiguous HBM. Pad rows are [1,0,...,0] with label-logit 0:
conf ~ e/(e+127) ~ 0.021 < 1/15 so they never cross any threshold, and
they are never counted correct, making them invisible to the output.
"""

import numpy as np

import concourse.bass as bass
import concourse.bacc as bacc
import concourse.tile as tile
from concourse import mybir
from concourse.bass_utils import run_bass_kernel_spmd

N_BINS = 15
C = 128
N_CORES = 8
P = 128          # SBUF partitions
Q = 992          # samples per partition per core (padded)
N_PAD = P * Q    # 126976 padded samples per core
CH = 62          # samples-per-partition per chunk (~2MB DMA)
N_CHUNKS = Q // CH  # 16

_F32 = mybir.dt.float32
_F16 = mybir.dt.float16


def _build_bass(q: int = Q, ch: int = CH) -> bass.Bass:
    n_chunks = q // ch
    nc = bacc.Bacc(None, target_bir_lowering=False)
    lg = nc.dram_tensor("logits", [P, q * C], _F16, kind="ExternalInput")
    xl = nc.dram_tensor("xl", [P, q], _F16, kind="ExternalInput")
    out = nc.dram_tensor("out", [P, 32], _F32, kind="ExternalOutput")

    # thresholds ln(b/15) computed from the same f32 linspace the reference uses
    lowers = np.linspace(0.0, 1.0, N_BINS + 1, dtype=np.float32)[:-1]
    thetas = [float(np.log(np.float64(lowers[b]))) for b in range(1, N_BINS)]

    AF = mybir.ActivationFunctionType
    OP = mybir.AluOpType

    with tile.TileContext(nc) as tc:
        with (
            tc.tile_pool(name="xin", bufs=3) as xpool,
            tc.tile_pool(name="eexp", bufs=2) as epool,
            tc.tile_pool(name="fold", bufs=1) as fpool,
            tc.tile_pool(name="wide", bufs=1) as wide,
        ):
            mx = wide.tile([P, q], _F32)      # per-sample max(x)
            sr = wide.tile([P, q], _F32)      # per-sample sum(exp(x))
            tt = wide.tile([P, q], _F32)      # t = mx - ln(sr)
            accv = wide.tile([P, q], _F32)    # correctness 0/1
            xl16 = wide.tile([P, q], _F16)    # per-sample x[label] (fp16)
            xl32 = wide.tile([P, q], _F32)
            mask = wide.tile([P, q], _F32)    # threshold mask scratch
            junk = wide.tile([P, q], _F32)    # scratch
            out_t = wide.tile([P, 32], _F32)

            nc.vector.memset(out_t[:], 0.0)
            nc.sync.dma_start(out=xl16[:], in_=xl[:])

            for j in range(n_chunks):
                xt = xpool.tile([P, ch, C], _F16)
                nc.sync.dma_start(out=xt[:], in_=lg[:, j * ch * C : (j + 1) * ch * C])
                et = epool.tile([P, ch, C], _F16)
                nc.scalar.activation(out=et[:], in_=xt[:], func=AF.Exp)

                # max fold tree on raw logits (fp16 tensor_tensor = 2x mode)
                m64 = fpool.tile([P, ch, 64], _F16)
                nc.vector.tensor_tensor(m64[:], xt[:, :, 0:64], xt[:, :, 64:128], OP.max)
                m32 = fpool.tile([P, ch, 32], _F16)
                nc.vector.tensor_tensor(m32[:], m64[:, :, 0:32], m64[:, :, 32:64], OP.max)
                m16 = fpool.tile([P, ch, 16], _F16)
                nc.vector.tensor_tensor(m16[:], m32[:, :, 0:16], m32[:, :, 16:32], OP.max)
                m8 = fpool.tile([P, ch, 8], _F16)
                nc.vector.tensor_tensor(m8[:], m16[:, :, 0:8], m16[:, :, 8:16], OP.max)
                nc.vector.tensor_reduce(
                    out=mx[:, j * ch : (j + 1) * ch], in_=m8[:],
                    axis=mybir.AxisListType.X, op=OP.max,
                )

                # sum fold tree on exp(x); final reduce accumulates in fp32
                s64 = fpool.tile([P, ch, 64], _F16)
                nc.vector.tensor_tensor(s64[:], et[:, :, 0:64], et[:, :, 64:128], OP.add)
                s32 = fpool.tile([P, ch, 32], _F16)
                nc.vector.tensor_tensor(s32[:], s64[:, :, 0:32], s64[:, :, 32:64], OP.add)
                s16 = fpool.tile([P, ch, 16], _F16)
                nc.vector.tensor_tensor(s16[:], s32[:, :, 0:16], s32[:, :, 16:32], OP.add)
                s8 = fpool.tile([P, ch, 8], _F16)
                nc.vector.tensor_tensor(s8[:], s16[:, :, 0:8], s16[:, :, 8:16], OP.add)
                nc.vector.tensor_reduce(
                    out=sr[:, j * ch : (j + 1) * ch], in_=s8[:],
                    axis=mybir.AxisListType.X, op=OP.add,
                )

            # ---- per-sample epilogue on [128, Q] wide tiles ----
            # t = mx - ln(sum e)  (log-confidence)
            nc.scalar.activation(out=junk[:], in_=sr[:], func=AF.Ln)
            nc.vector.tensor_tensor(tt[:], mx[:], junk[:], OP.subtract)
            # acc: x[label] >= max(x), on raw (quantized) values — exact compare
            nc.scalar.copy(out=xl32[:], in_=xl16[:])
            nc.vector.tensor_tensor(accv[:], xl32[:], mx[:], OP.is_ge)
            # col 16: total correct
            nc.vector.tensor_scalar(
                junk[:], accv[:], 1.0, None, OP.mult,
                op1=OP.add, accum_out=out_t[:, 16:17],
            )
            for b in range(1, N_BINS):
                # col b: ct_b = sum(t >= theta_b); op1 is the accum reduce op
                nc.vector.tensor_scalar(
                    mask[:], tt[:], thetas[b - 1], None, OP.is_ge,
                    op1=OP.add, accum_out=out_t[:, b : b + 1],
                )
                # col 16+b: cc_b = sum((t >= theta_b) * acc) in one fused op
                nc.vector.scalar_tensor_tensor(
                    out=junk[:], in0=tt[:], scalar=thetas[b - 1], in1=accv[:],
                    op0=OP.is_ge, op1=OP.mult,
                    accum_out=out_t[:, 16 + b : 17 + b],
                )
            nc.sync.dma_start(out=out[:], in_=out_t[:])
    nc.compile()
    return nc


_NC_CACHE = None


def _get_nc() -> bass.Bass:
    global _NC_CACHE
    if _NC_CACHE is None:
        _NC_CACHE = _build_bass()
    return _NC_CACHE


def make_in_maps(logits: np.ndarray, labels: np.ndarray):
    """Shard + pad full inputs into per-core fp16 input maps."""
    n = logits.shape[0]
    n_per = n // N_CORES
    assert n_per * N_CORES == n and n_per <= N_PAD
    lg16 = logits.astype(np.float16)
    idx = np.arange(n, dtype=np.int64)
    xl_full = lg16[idx, labels.astype(np.int64)]

    pad_row = np.zeros(C, np.float16)
    pad_row[0] = 1.0
    in_maps = []
    for s in range(N_CORES):
        lo = s * n_per
        shard = np.empty((N_PAD, C), np.float16)
        shard[:n_per] = lg16[lo : lo + n_per]
        shard[n_per:] = pad_row  # conf ~0.021 (bin 0), never correct
        xls = np.zeros(N_PAD, np.float16)
        xls[:n_per] = xl_full[lo : lo + n_per]
        in_maps.append(
            {"logits": shard.reshape(P, Q * C), "xl": xls.reshape(P, Q)}
        )
    return in_maps


def combine_outputs(results, n: int) -> np.ndarray:
    """Fold per-core [128,32] partials into the [15,2] bin_stats table."""
    ct = np.zeros(N_BINS + 1, np.float64)  # cumulative totals, index b
    cc = np.zeros(N_BINS + 1, np.float64)  # cumulative corrects
    ct[0] = float(n)
    for r in results:
        o = np.asarray(r["out"], np.float64)
        colsum = o.sum(axis=0)
        ct[1:N_BINS] += colsum[1:N_BINS]
        cc[0] += colsum[16]
        cc[1:N_BINS] += colsum[17 : 16 + N_BINS]
    total = ct[:-1] - ct[1:]
    correct = cc[:-1] - cc[1:]
    return np.stack([correct, total - correct], axis=1).astype(np.float32)


def kernel(logits, labels) -> np.ndarray:
    logits = np.asarray(logits, dtype=np.float32)
    labels = np.asarray(labels)
    n = logits.shape[0]
    in_maps = make_in_maps(logits, labels)
    res = run_bass_kernel_spmd(_get_nc(), in_maps, core_ids=list(range(N_CORES)))
    return combine_outputs(res.results, n)


# revision 9
# speedup vs baseline: 1.9961x; 1.0057x over previous
"""Confidence-histogram (ECE bin stats) Trainium2 Bass kernel.

Full-input contract: kernel(logits[1M,128] f32, labels[1M] int) -> [15,2] f32.

Math: conf = max(softmax(x)) = exp(mx) / S with mx = max_c x_c, S = sum_c
exp(x_c); prediction is correct iff x[label] == mx (ties are measure-zero
for randn inputs).  Binning via cumulative threshold counts in log space:
    t = mx - ln(S);   conf >= b/15  <=>  t >= ln(b/15)
Each core computes, for b in 1..14, ct_b = #{t >= theta_b} and
cc_b = #{t >= theta_b and correct}, plus cc_0 = total correct, as
per-partition partial sums. Host diffs the cumulative counts into the
[15,2] (correct, incorrect) table.

Inputs are cast to fp16 on the host (rel-err impact ~7e-5, gate is 2e-2),
which halves HBM traffic and unlocks the DVE 2x_1p mode for the pairwise
fold tree that replaces 1x per-sample reductions:
    max: [P,ch,128] -fold-> 64 -> 32 -> 16 -> 8 -then-> tensor_reduce(X)
    sum: same tree over exp(x) (ACT bulk exp, fp16 out)
Sharding: data-parallel over N across 8 cores; each 125k-sample shard is
padded to 128*992 rows laid out partition-major so every DMA descriptor
reads ~16KB contiguous HBM. Pad rows are [1,0,...,0] with label-logit 0:
conf ~ e/(e+127) ~ 0.021 < 1/15 so they never cross any threshold, and
they are never counted correct, making them invisible to the output.
"""

import numpy as np

import concourse.bass as bass
import concourse.bacc as bacc
import concourse.tile as tile
from concourse import mybir
from concourse.bass_utils import run_bass_kernel_spmd

N_BINS = 15
C = 128
N_CORES = 8
P = 128          # SBUF partitions
Q = 992          # samples per partition per core (padded)
N_PAD = P * Q    # 126976 padded samples per core
CH = 62          # samples-per-partition per chunk (~2MB DMA)
N_CHUNKS = Q // CH  # 16

_F32 = mybir.dt.float32
_F16 = mybir.dt.float16


def _build_bass(q: int = Q, ch: int = CH) -> bass.Bass:
    n_chunks = q // ch
    nc = bacc.Bacc(None, target_bir_lowering=False)
    lg = nc.dram_tensor("logits", [P, q * C], _F16, kind="ExternalInput")
    xl = nc.dram_tensor("xl", [P, q], _F16, kind="ExternalInput")
    out = nc.dram_tensor("out", [P, 32], _F32, kind="ExternalOutput")

    # thresholds ln(b/15) computed from the same f32 linspace the reference uses
    lowers = np.linspace(0.0, 1.0, N_BINS + 1, dtype=np.float32)[:-1]
    thetas = [float(np.log(np.float64(lowers[b]))) for b in range(1, N_BINS)]

    AF = mybir.ActivationFunctionType
    OP = mybir.AluOpType

    with tile.TileContext(nc) as tc:
        with (
            tc.tile_pool(name="xin", bufs=3) as xpool,
            tc.tile_pool(name="eexp", bufs=2) as epool,
            tc.tile_pool(name="fold", bufs=1) as fpool,
            tc.tile_pool(name="wide", bufs=1) as wide,
        ):
            mx = wide.tile([P, q], _F16)      # per-sample max(x) (fp16-exact)
            sr = wide.tile([P, q], _F16)      # per-sample sum(exp(x))
            tt = wide.tile([P, q], _F16)      # t = mx - ln(sr)
            accv = wide.tile([P, q], _F16)    # correctness 0/1
            xl16 = wide.tile([P, q], _F16)    # per-sample x[label] (fp16)
            mask = wide.tile([P, q], _F16)    # threshold mask scratch
            junk = wide.tile([P, q], _F32)    # scratch
            out_t = wide.tile([P, 32], _F32)

            nc.vector.memset(out_t[:], 0.0)
            nc.sync.dma_start(out=xl16[:], in_=xl[:])

            for j in range(n_chunks):
                xt = xpool.tile([P, ch, C], _F16)
                nc.sync.dma_start(out=xt[:], in_=lg[:, j * ch * C : (j + 1) * ch * C])
                et = epool.tile([P, ch, C], _F16)
                nc.scalar.activation(out=et[:], in_=xt[:], func=AF.Exp)

                # max fold tree on raw logits (fp16 tensor_tensor = 2x mode),
                # folded all the way to width 1 — cheaper than a 1x
                # tensor_reduce tail, and keeps mx fp16-exact for the
                # label-logit equality compare
                m64 = fpool.tile([P, ch, 64], _F16)
                nc.vector.tensor_tensor(m64[:], xt[:, :, 0:64], xt[:, :, 64:128], OP.max)
                m32 = fpool.tile([P, ch, 32], _F16)
                nc.vector.tensor_tensor(m32[:], m64[:, :, 0:32], m64[:, :, 32:64], OP.max)
                m16 = fpool.tile([P, ch, 16], _F16)
                nc.vector.tensor_tensor(m16[:], m32[:, :, 0:16], m32[:, :, 16:32], OP.max)
                m8 = fpool.tile([P, ch, 8], _F16)
                nc.vector.tensor_tensor(m8[:], m16[:, :, 0:8], m16[:, :, 8:16], OP.max)
                m4 = fpool.tile([P, ch, 4], _F16)
                nc.vector.tensor_tensor(m4[:], m8[:, :, 0:4], m8[:, :, 4:8], OP.max)
                m2 = fpool.tile([P, ch, 2], _F16)
                nc.vector.tensor_tensor(m2[:], m4[:, :, 0:2], m4[:, :, 2:4], OP.max)
                nc.vector.tensor_tensor(
                    mx[:, j * ch : (j + 1) * ch],
                    m2[:, :, 0:1], m2[:, :, 1:2], OP.max,
                )

                # sum fold tree on exp(x)
                s64 = fpool.tile([P, ch, 64], _F16)
                nc.vector.tensor_tensor(s64[:], et[:, :, 0:64], et[:, :, 64:128], OP.add)
                s32 = fpool.tile([P, ch, 32], _F16)
                nc.vector.tensor_tensor(s32[:], s64[:, :, 0:32], s64[:, :, 32:64], OP.add)
                s16 = fpool.tile([P, ch, 16], _F16)
                nc.vector.tensor_tensor(s16[:], s32[:, :, 0:16], s32[:, :, 16:32], OP.add)
                s8 = fpool.tile([P, ch, 8], _F16)
                nc.vector.tensor_tensor(s8[:], s16[:, :, 0:8], s16[:, :, 8:16], OP.add)
                s4 = fpool.tile([P, ch, 4], _F16)
                nc.vector.tensor_tensor(s4[:], s8[:, :, 0:4], s8[:, :, 4:8], OP.add)
                s2 = fpool.tile([P, ch, 2], _F16)
                nc.vector.tensor_tensor(s2[:], s4[:, :, 0:2], s4[:, :, 2:4], OP.add)
                nc.vector.tensor_tensor(
                    sr[:, j * ch : (j + 1) * ch],
                    s2[:, :, 0:1], s2[:, :, 1:2], OP.add,
                )

            # ---- per-sample epilogue on [128, Q] wide tiles ----
            # t = mx - ln(sum e)  (log-confidence), narrowed to fp16 so the
            # threshold count ops run in packed DVE mode
            nc.scalar.activation(out=junk[:], in_=sr[:], func=AF.Ln)
            nc.vector.tensor_tensor(tt[:], mx[:], junk[:], OP.subtract)
            # acc: x[label] >= max(x) — both fp16-exact, so the compare is exact
            nc.vector.tensor_tensor(accv[:], xl16[:], mx[:], OP.is_ge)
            # u = (t + SHIFT) * acc: correct samples keep t (shifted positive
            # for every bin edge), incorrect collapse to 0 < theta_1 + SHIFT,
            # so cc_b = #{u >= theta_b + SHIFT} has the same form as ct_b.
            SHIFT = 2.75  # > -ln(1/15) = 2.708
            uv = wide.tile([P, q], _F16)
            nc.vector.scalar_tensor_tensor(
                out=uv[:], in0=tt[:], scalar=SHIFT, in1=accv[:],
                op0=OP.add, op1=OP.mult,
            )
            # col 16: total correct
            nc.vector.tensor_scalar(
                mask[:], accv[:], 1.0, None, OP.mult,
                op1=OP.add, accum_out=out_t[:, 16:17],
            )
            for b in range(1, N_BINS):
                # col b: ct_b = sum(t >= theta_b); op1 is the accum reduce op
                nc.vector.tensor_scalar(
                    mask[:], tt[:], thetas[b - 1], None, OP.is_ge,
                    op1=OP.add, accum_out=out_t[:, b : b + 1],
                )
                # col 16+b: cc_b = sum(u >= theta_b + SHIFT)
                nc.vector.tensor_scalar(
                    mask[:], uv[:], thetas[b - 1] + SHIFT, None, OP.is_ge,
                    op1=OP.add, accum_out=out_t[:, 16 + b : 17 + b],
                )
            nc.sync.dma_start(out=out[:], in_=out_t[:])
    nc.compile()
    return nc


_NC_CACHE = None


def _get_nc() -> bass.Bass:
    global _NC_CACHE
    if _NC_CACHE is None:
        _NC_CACHE = _build_bass()
    return _NC_CACHE


def make_in_maps(logits: np.ndarray, labels: np.ndarray):
    """Shard + pad full inputs into per-core fp16 input maps."""
    n = logits.shape[0]
    n_per = n // N_CORES
    assert n_per * N_CORES == n and n_per <= N_PAD
    lg16 = logits.astype(np.float16)
    idx = np.arange(n, dtype=np.int64)
    xl_full = lg16[idx, labels.astype(np.int64)]

    pad_row = np.zeros(C, np.float16)
    pad_row[0] = 1.0
    in_maps = []
    for s in range(N_CORES):
        lo = s * n_per
        shard = np.empty((N_PAD, C), np.float16)
        shard[:n_per] = lg16[lo : lo + n_per]
        shard[n_per:] = pad_row  # conf ~0.021 (bin 0), never correct
        xls = np.zeros(N_PAD, np.float16)
        xls[:n_per] = xl_full[lo : lo + n_per]
        in_maps.append(
            {"logits": shard.reshape(P, Q * C), "xl": xls.reshape(P, Q)}
        )
    return in_maps


def combine_outputs(results, n: int) -> np.ndarray:
    """Fold per-core [128,32] partials into the [15,2] bin_stats table."""
    ct = np.zeros(N_BINS + 1, np.float64)  # cumulative totals, index b
    cc = np.zeros(N_BINS + 1, np.float64)  # cumulative corrects
    ct[0] = float(n)
    for r in results:
        o = np.asarray(r["out"], np.float64)
        colsum = o.sum(axis=0)
        ct[1:N_BINS] += colsum[1:N_BINS]
        cc[0] += colsum[16]
        cc[1:N_BINS] += colsum[17 : 16 + N_BINS]
    total = ct[:-1] - ct[1:]
    correct = cc[:-1] - cc[1:]
    return np.stack([correct, total - correct], axis=1).astype(np.float32)


def kernel(logits, labels) -> np.ndarray:
    logits = np.asarray(logits, dtype=np.float32)
    labels = np.asarray(labels)
    n = logits.shape[0]
    in_maps = make_in_maps(logits, labels)
    res = run_bass_kernel_spmd(_get_nc(), in_maps, core_ids=list(range(N_CORES)))
    return combine_outputs(res.results, n)


# revision 16
# speedup vs baseline: 2.1696x; 1.0869x over previous
"""Confidence-histogram (ECE bin stats) Trainium2 Bass kernel.

Full-input contract: kernel(logits[1M,128] f32, labels[1M] int) -> [15,2] f32.

Math: conf = max(softmax(x)) = exp(mx) / S with mx = max_c x_c, S = sum_c
exp(x_c); prediction is correct iff x[label] == mx (ties are measure-zero
for randn inputs).  Binning via cumulative threshold counts in log space:
    t = mx - ln(S);   conf >= b/15  <=>  t >= ln(b/15)
Each core computes, for b in 1..14, ct_b = #{t >= theta_b} and
cc_b = #{t >= theta_b and correct}, plus cc_0 = total correct, as
per-partition partial sums. Host diffs the cumulative counts into the
[15,2] (correct, incorrect) table.

Inputs are cast to fp16 on the host (rel-err impact ~7e-5, gate is 2e-2),
which halves HBM traffic and unlocks the DVE 2x_1p mode for the pairwise
fold tree that replaces 1x per-sample reductions:
    max: [P,ch,128] -fold-> 64 -> 32 -> 16 -> 8 -then-> tensor_reduce(X)
    sum: same tree over exp(x) (ACT bulk exp, fp16 out)
Sharding: data-parallel over N across 8 cores; each 125k-sample shard is
padded to 128*992 rows laid out partition-major so every DMA descriptor
reads ~16KB contiguous HBM. Pad rows are [1,0,...,0] with label-logit 0:
conf ~ e/(e+127) ~ 0.021 < 1/15 so they never cross any threshold, and
they are never counted correct, making them invisible to the output.
"""

import numpy as np

import concourse.bass as bass
import concourse.bacc as bacc
import concourse.tile as tile
from concourse import mybir
from concourse.bass_utils import run_bass_kernel_spmd

N_BINS = 15
C = 128
N_CORES = 8
P = 128          # SBUF partitions
Q = 992          # samples per partition per core (padded)
N_PAD = P * Q    # 126976 padded samples per core
CH = 62          # samples-per-partition per chunk (~2MB DMA)
N_CHUNKS = Q // CH  # 16

_F32 = mybir.dt.float32
_F16 = mybir.dt.float16


def _build_bass(q: int = Q, ch: int = CH) -> bass.Bass:
    n_chunks = q // ch
    nc = bacc.Bacc(None, target_bir_lowering=False)
    lg = nc.dram_tensor("logits", [P, q * C], _F16, kind="ExternalInput")
    xl = nc.dram_tensor("xl", [P, q], _F16, kind="ExternalInput")
    out = nc.dram_tensor("out", [P, 32], _F32, kind="ExternalOutput")

    # thresholds ln(b/15) computed from the same f32 linspace the reference uses
    lowers = np.linspace(0.0, 1.0, N_BINS + 1, dtype=np.float32)[:-1]
    thetas = [float(np.log(np.float64(lowers[b]))) for b in range(1, N_BINS)]

    AF = mybir.ActivationFunctionType
    OP = mybir.AluOpType

    with tile.TileContext(nc) as tc:
        with (
            tc.tile_pool(name="xin", bufs=3) as xpool,
            tc.tile_pool(name="eexp", bufs=2) as epool,
            tc.tile_pool(name="fold", bufs=1) as fpool,
            tc.tile_pool(name="wide", bufs=1) as wide,
        ):
            mx = wide.tile([P, q], _F16)      # per-sample max(x) (fp16-exact)
            sr = wide.tile([P, q], _F16)      # per-sample sum(exp(x))
            tt = wide.tile([P, q], _F16)      # t = mx - ln(sr)
            accv = wide.tile([P, q], _F16)    # correctness 0/1
            xl16 = wide.tile([P, q], _F16)    # per-sample x[label] (fp16)
            mask = wide.tile([P, q], _F16)    # threshold mask scratch
            junk = wide.tile([P, q], _F32)    # scratch
            out_t = wide.tile([P, 32], _F32)

            nc.vector.memset(out_t[:], 0.0)
            # bias consts for the ACT sign-count ops ([P,1] APs, built on the
            # idle GPSIMD engine during the first chunk's DMA)
            SHIFT = 2.75  # > -ln(1/15) = 2.708
            bias_t = wide.tile([P, 2 * (N_BINS - 1)], _F32)
            for b in range(1, N_BINS):
                nc.gpsimd.memset(bias_t[:, b - 1 : b], -thetas[b - 1])
                nc.gpsimd.memset(
                    bias_t[:, 13 + b : 14 + b], -(thetas[b - 1] + SHIFT)
                )

            for j in range(n_chunks):
                xt = xpool.tile([P, ch, C], _F16)
                nc.sync.dma_start(out=xt[:], in_=lg[:, j * ch * C : (j + 1) * ch * C])
                if j == 1:
                    # xl is only needed at the epilogue — don't let it delay
                    # the first logits chunk
                    nc.sync.dma_start(out=xl16[:], in_=xl[:])
                et = epool.tile([P, ch, C], _F16)
                nc.scalar.activation(out=et[:], in_=xt[:], func=AF.Exp)

                # max fold tree on raw logits (fp16 tensor_tensor = 2x mode),
                # folded all the way to width 1 — cheaper than a 1x
                # tensor_reduce tail, and keeps mx fp16-exact for the
                # label-logit equality compare
                m64 = fpool.tile([P, ch, 64], _F16)
                nc.vector.tensor_tensor(m64[:], xt[:, :, 0:64], xt[:, :, 64:128], OP.max)
                m32 = fpool.tile([P, ch, 32], _F16)
                nc.vector.tensor_tensor(m32[:], m64[:, :, 0:32], m64[:, :, 32:64], OP.max)
                m16 = fpool.tile([P, ch, 16], _F16)
                nc.vector.tensor_tensor(m16[:], m32[:, :, 0:16], m32[:, :, 16:32], OP.max)
                m8 = fpool.tile([P, ch, 8], _F16)
                nc.vector.tensor_tensor(m8[:], m16[:, :, 0:8], m16[:, :, 8:16], OP.max)
                m4 = fpool.tile([P, ch, 4], _F16)
                nc.vector.tensor_tensor(m4[:], m8[:, :, 0:4], m8[:, :, 4:8], OP.max)
                m2 = fpool.tile([P, ch, 2], _F16)
                nc.vector.tensor_tensor(m2[:], m4[:, :, 0:2], m4[:, :, 2:4], OP.max)
                nc.vector.tensor_tensor(
                    mx[:, j * ch : (j + 1) * ch],
                    m2[:, :, 0:1], m2[:, :, 1:2], OP.max,
                )

                # sum fold tree on exp(x)
                s64 = fpool.tile([P, ch, 64], _F16)
                nc.vector.tensor_tensor(s64[:], et[:, :, 0:64], et[:, :, 64:128], OP.add)
                s32 = fpool.tile([P, ch, 32], _F16)
                nc.vector.tensor_tensor(s32[:], s64[:, :, 0:32], s64[:, :, 32:64], OP.add)
                s16 = fpool.tile([P, ch, 16], _F16)
                nc.vector.tensor_tensor(s16[:], s32[:, :, 0:16], s32[:, :, 16:32], OP.add)
                s8 = fpool.tile([P, ch, 8], _F16)
                nc.vector.tensor_tensor(s8[:], s16[:, :, 0:8], s16[:, :, 8:16], OP.add)
                s4 = fpool.tile([P, ch, 4], _F16)
                nc.vector.tensor_tensor(s4[:], s8[:, :, 0:4], s8[:, :, 4:8], OP.add)
                s2 = fpool.tile([P, ch, 2], _F16)
                nc.vector.tensor_tensor(s2[:], s4[:, :, 0:2], s4[:, :, 2:4], OP.add)
                nc.vector.tensor_tensor(
                    sr[:, j * ch : (j + 1) * ch],
                    s2[:, :, 0:1], s2[:, :, 1:2], OP.add,
                )

            # ---- per-sample epilogue on [128, Q] wide tiles ----
            # t = mx - ln(sum e)  (log-confidence), narrowed to fp16 so the
            # threshold count ops run in packed DVE mode
            nc.scalar.activation(out=junk[:], in_=sr[:], func=AF.Ln)
            nc.vector.tensor_tensor(tt[:], mx[:], junk[:], OP.subtract)
            # acc: x[label] >= max(x) — both fp16-exact, so the compare is exact
            nc.vector.tensor_tensor(accv[:], xl16[:], mx[:], OP.is_ge)
            # u = (t + SHIFT) * acc: correct samples keep t (shifted positive
            # for every bin edge), incorrect collapse to 0 < theta_1 + SHIFT,
            # so cc_b = #{u >= theta_b + SHIFT} has the same form as ct_b.
            uv = wide.tile([P, q], _F16)
            nc.vector.scalar_tensor_tensor(
                out=uv[:], in0=tt[:], scalar=SHIFT, in1=accv[:],
                op0=OP.add, op1=OP.mult,
            )
            # col 16: total correct
            nc.vector.tensor_scalar(
                mask[:], accv[:], 1.0, None, OP.mult,
                op1=OP.add, accum_out=out_t[:, 16:17],
            )
            # Threshold counts split across the two idle-at-tail engines:
            # DVE counts via is_ge+add accum (exact 0/1 sums); ACT counts via
            # sign(t - theta) + accum — sum(sign) = 2*ct - Q, decoded on the
            # host (marked by out_t[p, 31] = 1 convention: odd b on ACT).
            junk16 = wide.tile([P, q], _F16)
            for b in range(1, N_BINS):
                # col b: ct_b = #{t >= theta_b}
                if b % 2 == 0:
                    nc.vector.tensor_scalar(
                        mask[:], tt[:], thetas[b - 1], None, OP.is_ge,
                        op1=OP.add, accum_out=out_t[:, b : b + 1],
                    )
                else:
                    nc.scalar.activation(
                        out=junk16[:], in_=tt[:], func=AF.Sign,
                        bias=bias_t[:, b - 1 : b],
                        accum_out=out_t[:, b : b + 1],
                    )
                # col 16+b: cc_b = #{u >= theta_b + SHIFT}
                if b % 2 == 1:
                    nc.vector.tensor_scalar(
                        mask[:], uv[:], thetas[b - 1] + SHIFT, None, OP.is_ge,
                        op1=OP.add, accum_out=out_t[:, 16 + b : 17 + b],
                    )
                else:
                    nc.scalar.activation(
                        out=junk16[:], in_=uv[:], func=AF.Sign,
                        bias=bias_t[:, 13 + b : 14 + b],
                        accum_out=out_t[:, 16 + b : 17 + b],
                    )
            nc.sync.dma_start(out=out[:], in_=out_t[:])
    nc.compile()
    return nc


_NC_CACHE = None


def _get_nc() -> bass.Bass:
    global _NC_CACHE
    if _NC_CACHE is None:
        _NC_CACHE = _build_bass()
    return _NC_CACHE


def make_in_maps(logits: np.ndarray, labels: np.ndarray):
    """Shard + pad full inputs into per-core fp16 input maps."""
    n = logits.shape[0]
    n_per = n // N_CORES
    assert n_per * N_CORES == n and n_per <= N_PAD
    lg16 = logits.astype(np.float16)
    idx = np.arange(n, dtype=np.int64)
    xl_full = lg16[idx, labels.astype(np.int64)]

    pad_row = np.zeros(C, np.float16)
    pad_row[0] = 1.0
    in_maps = []
    for s in range(N_CORES):
        lo = s * n_per
        shard = np.empty((N_PAD, C), np.float16)
        shard[:n_per] = lg16[lo : lo + n_per]
        shard[n_per:] = pad_row  # conf ~0.021 (bin 0), never correct
        xls = np.zeros(N_PAD, np.float16)
        xls[:n_per] = xl_full[lo : lo + n_per]
        in_maps.append(
            {"logits": shard.reshape(P, Q * C), "xl": xls.reshape(P, Q)}
        )
    return in_maps


def combine_outputs(results, n: int) -> np.ndarray:
    """Fold per-core [128,32] partials into the [15,2] bin_stats table.

    Columns counted on the ACT engine hold sum(sign(x - theta)) = 2*count - T
    (T = all padded samples across cores); decode those back to counts.
    ct cols use ACT for odd b, cc cols for even b.
    """
    tot = float(N_CORES * N_PAD)
    colsum = np.zeros(32, np.float64)
    for r in results:
        colsum += np.asarray(r["out"], np.float64).sum(axis=0)
    ct = np.zeros(N_BINS + 1, np.float64)  # cumulative totals, index b
    cc = np.zeros(N_BINS + 1, np.float64)  # cumulative corrects
    ct[0] = float(n)
    cc[0] = colsum[16]
    for b in range(1, N_BINS):
        raw_ct = colsum[b]
        ct[b] = (raw_ct + tot) / 2.0 if b % 2 == 1 else raw_ct
        raw_cc = colsum[16 + b]
        cc[b] = (raw_cc + tot) / 2.0 if b % 2 == 0 else raw_cc
    total = ct[:-1] - ct[1:]
    correct = cc[:-1] - cc[1:]
    return np.stack([correct, total - correct], axis=1).astype(np.float32)


def kernel(logits, labels) -> np.ndarray:
    logits = np.asarray(logits, dtype=np.float32)
    labels = np.asarray(labels)
    n = logits.shape[0]
    in_maps = make_in_maps(logits, labels)
    res = run_bass_kernel_spmd(_get_nc(), in_maps, core_ids=list(range(N_CORES)))
    return combine_outputs(res.results, n)
